# revision 42
# baseline (speedup 1.0000x reference)
"""Trainium2 Bass kernel for the Diversity8 loss.

loss = SCALE * mean_b d[b],   d[b] = (||sum_m v_m[b]||^2 - M) / 2
where v_m[b] = unit-normalized, mean-centered softmax(logits_m[b]/T).

Softmax centering + normalization are shift/scale invariant, so
v_m = (e - mean e) / ||e - mean e||  with  e = exp(x/T).

Per (model m, 128-sample group), all stats per partition row:
  - ACT:  e = Exp(x/T), accum_out -> S = sum e
  - DVE:  ec = e - S/C  (centered tile, written as float32r)
  - var:  m2 = sum ec^2, split between ACT Square and DVE
    scalar_tensor_tensor for engine balance.  The SQUARED form is
    load-bearing: sum (e-eb)*e leaks the accumulated-S rounding bias
    linearly into alpha, which biases the near-cancelling loss mean.
  - DVE:  alpha = rsqrt(m2) via Newton from a linear seed (no ACT
    table switch; one-sided convergence means iterations must run to
    fp32 convergence or alpha is biased low)
  - PE :  s += diag(alpha_m) @ ec_m accumulated over models in PSUM
    (float32r single-pass matmuls; centering BEFORE the matmul keeps
    the PE's reduced-precision product noise relative to |s|~0.1 --
    elementwise noise delta on s biases d by C*var(delta), chi^2)
  - ACT:  R = sum s^2 via Square with accum_out;  d = 0.5*R - M/2

Sharding: pure data parallel over the batch dim, 512 samples per core on
8 cores; host sums the per-core [128, 4] d-columns.
"""

import os
import sys

import numpy as np

for _p in ("/opt/trn_rl_repo", "/root/.axon_site/_ro/trn_rl_repo"):
    if os.path.isdir(_p) and _p not in sys.path:
        sys.path.append(_p)

import concourse.bacc as bacc
import concourse.mybir as mybir
from concourse import bass_utils
from concourse.tile import TileContext

F32 = mybir.dt.float32
F32R = mybir.dt.float32r
AF = mybir.ActivationFunctionType
OP = mybir.AluOpType

B = 4096
C = 1000
M = 8
T = 20.0
SCALE = 0.3
N_CORES = 8
B_SHARD = B // N_CORES          # 512 samples per core
G = B_SHARD // 128              # 4 groups of 128 samples
NEWTON_ITERS = 2
# minimax linear seed y0 = SEED_A + SEED_B*m2 on m2 in [2.0, 3.1]
# (equal-ripple |eps0| <= ~1%), so 2 Newton iters converge to fp32-exact
# with sign-balanced residual (one-sided-convergence bias avoided)
SEED_A = 0.95441
SEED_B = -0.12650
# per-group ACT-variance model count (rest on DVE): later groups lean on
# ACT because it has run out of exp work while DVE drains the tail
ACT_VAR_COUNT = (1, 2, 2, 4)
PSUM_SPLITS = ((0, 512), (512, 1000))
MM_DT = F32R                    # matmul operand dtype (full-speed fp32)

_cached = {}


def _build():
    nc = bacc.Bacc("TRN2", target_bir_lowering=False, debug=False)
    xs = [
        nc.dram_tensor(f"x{m}", [B_SHARD, C], F32, kind="ExternalInput")
        for m in range(M)
    ]
    ident_dram = nc.dram_tensor("ident", [128, 128], F32,
                                kind="ExternalInput")
    d_dram = nc.dram_tensor("d", [128, G], F32, kind="ExternalOutput")

    with TileContext(nc, pool_alloc_mode="queue") as tc:
        with (
            tc.tile_pool(name="const", bufs=1) as const_pool,
            tc.tile_pool(name="x", bufs=6) as x_pool,
            tc.tile_pool(name="e", bufs=12) as e_pool,
            tc.tile_pool(name="ec", bufs=14) as ec_pool,
            tc.tile_pool(name="w", bufs=6) as w_pool,
            tc.tile_pool(name="scr", bufs=4) as scr_pool,
            tc.tile_pool(name="stat", bufs=2) as stat_pool,
            tc.tile_pool(name="dout", bufs=1) as dout_pool,
            tc.tile_pool(name="psum", bufs=3, space="PSUM") as psum_pool,
            tc.tile_pool(name="wps", bufs=2, space="PSUM") as warm_pool,
        ):
            ident = const_pool.tile([128, 128], MM_DT)
            nc.sync.dma_start(out=ident[:, :],
                              in_=ident_dram.ap().bitcast(F32R))

            dout = dout_pool.tile([128, G], F32)
            rall = dout_pool.tile([128, G], F32)

            H = M // 2           # alpha computed per half-group of models
            for g in range(G):
                r0 = g * 128
                s_all = stat_pool.tile([128, M], F32)    # sum e per model
                negm = stat_pool.tile([128, M], F32)     # -mean e
                m2 = stat_pool.tile([128, M], F32)       # sum (e-ebar)^2
                alpha = stat_pool.tile([128, M], F32)
                tn = stat_pool.tile([128, M], F32)
                e_tiles = {}
                ec_tiles = {}
                n_act_var = ACT_VAR_COUNT[g]
                s_ps = psum_pool.tile([128, C], F32)
                segments = ((0, H), (H, M))
                k = (n_act_var + 1) // 2
                act_var_set = tuple(range(0, k)) + tuple(
                    range(H, H + n_act_var - k))
                for h, (h0, h1) in enumerate(segments):
                    for m in range(h0, h1):
                        warm_t = warm_pool.tile([2, 2], F32)
                        nc.tensor.matmul(warm_t[:, :], ident[:, :2],
                                         ident[:, :2], start=True, stop=True,
                                         skip_group_check=True)
                        x_t = x_pool.tile([128, C], F32)
                        # group 0 first half: issue via the ACT sequencer
                        # (HWDGE too) -- its preamble ends ~1.5us before
                        # Sync's, and ACT idles until the first exp anyway
                        dma_eng = nc.scalar if (g == 0 and h == 0) else nc.sync
                        dma_eng.dma_start(out=x_t[:, :],
                                          in_=xs[m].ap()[r0:r0 + 128, :])
                        e_t = e_pool.tile([128, C], F32)
                        nc.scalar.activation(e_t[:, :], x_t[:, :], AF.Exp,
                                             scale=1.0 / T,
                                             accum_out=s_all[:, m:m + 1])
                        e_tiles[m] = e_t
                        if g == 0 and h == 0:
                            nc.vector.tensor_scalar(negm[:, m:m + 1],
                                                    s_all[:, m:m + 1],
                                                    -1.0 / C, None, OP.mult)
                    if g == 0 and h == 0:
                        # per-model negm in the very first half-group: lets
                        # the DVE start centering after ONE exp instead of
                        # four (the kernel head is exp-latency-bound)
                        pass
                    else:
                        nc.vector.tensor_scalar(negm[:, h0:h1],
                                                s_all[:, h0:h1],
                                                -1.0 / C, None, OP.mult)
                    for m in range(h0, h1):
                        ec_t = ec_pool.tile([128, C], MM_DT)
                        nc.vector.tensor_scalar(ec_t[:, :],
                                                e_tiles[m][:, :],
                                                negm[:, m:m + 1], None,
                                                OP.add)
                        scr = scr_pool.tile([128, C], F32)
                        if m in act_var_set:
                            nc.scalar.activation(scr[:, :], ec_t[:, :],
                                                 AF.Square,
                                                 accum_out=m2[:, m:m + 1])
                        else:
                            nc.vector.scalar_tensor_tensor(
                                scr[:, :], ec_t[:, :], 0.0, ec_t[:, :],
                                op0=OP.bypass, op1=OP.mult,
                                accum_out=m2[:, m:m + 1])
                        ec_tiles[m] = ec_t

                    # last group: alpha + matmuls per HALF so the final
                    # PE burst starts while the second half's variance is
                    # still in flight (shrinks the exposed kernel tail)
                    split_tail = (g == G - 1)
                    if split_tail or h == 1:
                        hs = slice(h0, h1) if split_tail else slice(0, M)
                        nc.vector.tensor_scalar(alpha[:, hs], m2[:, hs],
                                                SEED_B, SEED_A,
                                                OP.mult, OP.add)
                        for _ in range(NEWTON_ITERS):
                            nc.vector.scalar_tensor_tensor(
                                tn[:, hs], alpha[:, hs], 0.0, alpha[:, hs],
                                op0=OP.bypass, op1=OP.mult)    # y^2
                            nc.vector.scalar_tensor_tensor(
                                tn[:, hs], tn[:, hs], -0.5, m2[:, hs],
                                op0=OP.mult, op1=OP.mult)      # -.5*y^2*m2
                            nc.vector.scalar_tensor_tensor(
                                alpha[:, hs], tn[:, hs], 1.5, alpha[:, hs],
                                op0=OP.add, op1=OP.mult)       # y*(1.5+t)
                        mms = (range(h0, h1) if split_tail
                               else range(M))
                        for m in mms:
                            w_t = w_pool.tile([128, 128], MM_DT)
                            nc.vector.tensor_scalar(w_t[:, :], ident[:, :],
                                                    alpha[:, m:m + 1], None,
                                                    OP.mult)
                            for c0, c1 in PSUM_SPLITS:
                                nc.tensor.matmul(s_ps[:, c0:c1],
                                                 w_t[:, :],
                                                 ec_tiles[m][:, c0:c1],
                                                 start=(m == 0),
                                                 stop=(m == M - 1))

                # R = sum s^2
                scr2 = scr_pool.tile([128, C], F32)
                nc.scalar.activation(scr2[:, :], s_ps[:, :], AF.Square,
                                     accum_out=rall[:, g:g + 1])

            # d = 0.5 * R - M/2 for all groups at once
            nc.vector.tensor_scalar(dout[:, :], rall[:, :],
                                    0.5, -M / 2.0, OP.mult, OP.add)
            nc.sync.dma_start(out=d_dram.ap(), in_=dout[:, :])

    nc.compile()
    return nc


def _get_nc():
    if "nc" not in _cached:
        _cached["nc"] = _build()
    return _cached["nc"]


_IDENT = np.eye(128, dtype=np.float32)


def kernel(**inputs: np.ndarray) -> np.ndarray:
    nc = _get_nc()
    outs = [np.asarray(inputs[f"outputs{m + 1}"], dtype=np.float32)
            for m in range(M)]
    in_maps = []
    for c in range(N_CORES):
        sl = slice(c * B_SHARD, (c + 1) * B_SHARD)
        im = {f"x{m}": np.ascontiguousarray(outs[m][sl]) for m in range(M)}
        im["ident"] = _IDENT
        in_maps.append(im)
    res = bass_utils.run_bass_kernel_spmd(nc, in_maps,
                                          core_ids=list(range(N_CORES)))
    total = 0.0
    for c in range(N_CORES):
        total += float(res.results[c]["d"].astype(np.float64).sum())
    return np.array(SCALE * total / B, dtype=np.float32)


# revision 43
# speedup vs baseline: 1.0599x; 1.0599x over previous
"""Trainium2 Bass kernel for the Diversity8 loss.

loss = SCALE * mean_b d[b],   d[b] = (||sum_m v_m[b]||^2 - M) / 2
where v_m[b] = unit-normalized, mean-centered softmax(logits_m[b]/T).

Softmax centering + normalization are shift/scale invariant, so
v_m = (e - mean e) / ||e - mean e||  with  e = exp(x/T).

Per (model m, 128-sample group), all stats per partition row:
  - ACT:  e = Exp(x/T), accum_out -> S = sum e
  - DVE:  ec = e - S/C  (centered tile, written as float32r)
  - var:  m2 = sum ec^2, split between ACT Square and DVE
    scalar_tensor_tensor for engine balance.  The SQUARED form is
    load-bearing: sum (e-eb)*e leaks the accumulated-S rounding bias
    linearly into alpha, which biases the near-cancelling loss mean.
  - DVE:  alpha = rsqrt(m2) via Newton from a linear seed (no ACT
    table switch; one-sided convergence means iterations must run to
    fp32 convergence or alpha is biased low)
  - PE :  s += diag(alpha_m) @ ec_m accumulated over models in PSUM
    (float32r single-pass matmuls; centering BEFORE the matmul keeps
    the PE's reduced-precision product noise relative to |s|~0.1 --
    elementwise noise delta on s biases d by C*var(delta), chi^2)
  - ACT:  R = sum s^2 via Square with accum_out;  d = 0.5*R - M/2

Sharding: pure data parallel over the batch dim, 512 samples per core on
8 cores; host sums the per-core [128, 4] d-columns.
"""

import os
import sys

import numpy as np

for _p in ("/opt/trn_rl_repo", "/root/.axon_site/_ro/trn_rl_repo"):
    if os.path.isdir(_p) and _p not in sys.path:
        sys.path.append(_p)

import concourse.bacc as bacc
import concourse.mybir as mybir
from concourse import bass_utils
from concourse.tile import TileContext

F32 = mybir.dt.float32
F32R = mybir.dt.float32r
AF = mybir.ActivationFunctionType
OP = mybir.AluOpType

B = 4096
C = 1000
M = 8
T = 20.0
SCALE = 0.3
N_CORES = 8
B_SHARD = B // N_CORES          # 512 samples per core
G = B_SHARD // 128              # 4 groups of 128 samples
NEWTON_ITERS = 2
# minimax linear seed y0 = SEED_A + SEED_B*m2 on m2 in [2.0, 3.1]
# (equal-ripple |eps0| <= ~1%), so 2 Newton iters converge to fp32-exact
# with sign-balanced residual (one-sided-convergence bias avoided)
SEED_A = 0.95441
SEED_B = -0.12650
# per-group ACT-variance model count (rest on DVE): later groups lean on
# ACT because it has run out of exp work while DVE drains the tail
ACT_VAR_COUNT = (1, 2, 2, 4)
PSUM_SPLITS = ((0, 512), (512, 1000))
MM_DT = F32R                    # matmul operand dtype (full-speed fp32)

_cached = {}


def _build():
    nc = bacc.Bacc("TRN2", target_bir_lowering=False, debug=False)
    xs = [
        nc.dram_tensor(f"x{m}", [B_SHARD, C], F32, kind="ExternalInput")
        for m in range(M)
    ]
    ident_dram = nc.dram_tensor("ident", [128, 128], F32,
                                kind="ExternalInput")
    d_dram = nc.dram_tensor("d", [128, G], F32, kind="ExternalOutput")

    with TileContext(nc, pool_alloc_mode="queue") as tc:
        with (
            tc.tile_pool(name="const", bufs=1) as const_pool,
            tc.tile_pool(name="x", bufs=6) as x_pool,
            tc.tile_pool(name="e", bufs=12) as e_pool,
            tc.tile_pool(name="ec", bufs=14) as ec_pool,
            tc.tile_pool(name="w", bufs=6) as w_pool,
            tc.tile_pool(name="scr", bufs=4) as scr_pool,
            tc.tile_pool(name="stat", bufs=2) as stat_pool,
            tc.tile_pool(name="dout", bufs=1) as dout_pool,
            tc.tile_pool(name="psum", bufs=3, space="PSUM") as psum_pool,
            tc.tile_pool(name="wps", bufs=2, space="PSUM") as warm_pool,
        ):
            ident = const_pool.tile([128, 128], MM_DT)
            nc.sync.dma_start(out=ident[:, :],
                              in_=ident_dram.ap().bitcast(F32R))

            dout = dout_pool.tile([128, G], F32)
            rall = dout_pool.tile([128, G], F32)

            H = M // 2           # alpha computed per half-group of models
            for g in range(G):
                r0 = g * 128
                s_all = stat_pool.tile([128, M], F32)    # sum e per model
                negm = stat_pool.tile([128, M], F32)     # -mean e
                m2 = stat_pool.tile([128, M], F32)       # sum (e-ebar)^2
                alpha = stat_pool.tile([128, M], F32)
                tn = stat_pool.tile([128, M], F32)
                e_tiles = {}
                ec_tiles = {}
                n_act_var = ACT_VAR_COUNT[g]
                s_ps = psum_pool.tile([128, C], F32)
                segments = ((0, H), (H, M))
                k = (n_act_var + 1) // 2
                act_var_set = tuple(range(0, k)) + tuple(
                    range(H, H + n_act_var - k))
                for h, (h0, h1) in enumerate(segments):
                    for m in range(h0, h1):
                        warm_t = warm_pool.tile([2, 2], F32)
                        nc.tensor.matmul(warm_t[:, :], ident[:, :2],
                                         ident[:, :2], start=True, stop=True,
                                         skip_group_check=True)
                        x_t = x_pool.tile([128, C], F32)
                        nc.sync.dma_start(out=x_t[:, :],
                                          in_=xs[m].ap()[r0:r0 + 128, :])
                        e_t = e_pool.tile([128, C], F32)
                        nc.scalar.activation(e_t[:, :], x_t[:, :], AF.Exp,
                                             scale=1.0 / T,
                                             accum_out=s_all[:, m:m + 1])
                        e_tiles[m] = e_t
                        if g == 0 and h == 0:
                            nc.vector.tensor_scalar(negm[:, m:m + 1],
                                                    s_all[:, m:m + 1],
                                                    -1.0 / C, None, OP.mult)
                    if g == 0 and h == 0:
                        # per-model negm in the very first half-group: lets
                        # the DVE start centering after ONE exp instead of
                        # four (the kernel head is exp-latency-bound)
                        pass
                    else:
                        nc.vector.tensor_scalar(negm[:, h0:h1],
                                                s_all[:, h0:h1],
                                                -1.0 / C, None, OP.mult)
                    for m in range(h0, h1):
                        ec_t = ec_pool.tile([128, C], MM_DT)
                        nc.vector.tensor_scalar(ec_t[:, :],
                                                e_tiles[m][:, :],
                                                negm[:, m:m + 1], None,
                                                OP.add)
                        scr = scr_pool.tile([128, C], F32)
                        if m in act_var_set:
                            nc.scalar.activation(scr[:, :], ec_t[:, :],
                                                 AF.Square,
                                                 accum_out=m2[:, m:m + 1])
                        else:
                            nc.vector.scalar_tensor_tensor(
                                scr[:, :], ec_t[:, :], 0.0, ec_t[:, :],
                                op0=OP.bypass, op1=OP.mult,
                                accum_out=m2[:, m:m + 1])
                        ec_tiles[m] = ec_t

                    # last group: alpha + matmuls per HALF so the final
                    # PE burst starts while the second half's variance is
                    # still in flight (shrinks the exposed kernel tail)
                    split_tail = (g == G - 1)
                    if split_tail or h == 1:
                        hs = slice(h0, h1) if split_tail else slice(0, M)
                        nc.vector.tensor_scalar(alpha[:, hs], m2[:, hs],
                                                SEED_B, SEED_A,
                                                OP.mult, OP.add)
                        for _ in range(NEWTON_ITERS):
                            nc.vector.scalar_tensor_tensor(
                                tn[:, hs], alpha[:, hs], 0.0, alpha[:, hs],
                                op0=OP.bypass, op1=OP.mult)    # y^2
                            nc.vector.scalar_tensor_tensor(
                                tn[:, hs], tn[:, hs], -0.5, m2[:, hs],
                                op0=OP.mult, op1=OP.mult)      # -.5*y^2*m2
                            nc.vector.scalar_tensor_tensor(
                                alpha[:, hs], tn[:, hs], 1.5, alpha[:, hs],
                                op0=OP.add, op1=OP.mult)       # y*(1.5+t)
                        mms = (range(h0, h1) if split_tail
                               else range(M))
                        for m in mms:
                            w_t = w_pool.tile([128, 128], MM_DT)
                            nc.vector.tensor_scalar(w_t[:, :], ident[:, :],
                                                    alpha[:, m:m + 1], None,
                                                    OP.mult)
                            for c0, c1 in PSUM_SPLITS:
                                nc.tensor.matmul(s_ps[:, c0:c1],
                                                 w_t[:, :],
                                                 ec_tiles[m][:, c0:c1],
                                                 start=(m == 0),
                                                 stop=(m == M - 1))

                # R = sum s^2
                scr2 = scr_pool.tile([128, C], F32)
                nc.scalar.activation(scr2[:, :], s_ps[:, :], AF.Square,
                                     accum_out=rall[:, g:g + 1])

            # d = 0.5 * R - M/2 for all groups at once
            nc.vector.tensor_scalar(dout[:, :], rall[:, :],
                                    0.5, -M / 2.0, OP.mult, OP.add)
            nc.sync.dma_start(out=d_dram.ap(), in_=dout[:, :])

    nc.compile()
    return nc


def _get_nc():
    if "nc" not in _cached:
        _cached["nc"] = _build()
    return _cached["nc"]


_IDENT = np.eye(128, dtype=np.float32)


def kernel(**inputs: np.ndarray) -> np.ndarray:
    nc = _get_nc()
    outs = [np.asarray(inputs[f"outputs{m + 1}"], dtype=np.float32)
            for m in range(M)]
    in_maps = []
    for c in range(N_CORES):
        sl = slice(c * B_SHARD, (c + 1) * B_SHARD)
        im = {f"x{m}": np.ascontiguousarray(outs[m][sl]) for m in range(M)}
        im["ident"] = _IDENT
        in_maps.append(im)
    res = bass_utils.run_bass_kernel_spmd(nc, in_maps,
                                          core_ids=list(range(N_CORES)))
    total = 0.0
    for c in range(N_CORES):
        total += float(res.results[c]["d"].astype(np.float64).sum())
    return np.array(SCALE * total / B, dtype=np.float32)
